# revision 20
# baseline (speedup 1.0000x reference)
"""Trainium2 Bass kernel for nn_CrossAttention (16x6209x256 cross-attention).

Strategy
--------
Data-parallel over batch: 16 batches -> 8 cores x 2 batches, pure SPMD.

Per batch:
    mapped_a = a @ Wa + ba            [6209, 64]
    mapped_b = b @ Wb + bb            [256, 64]
    scores   = mapped_a @ mapped_b.T * 8
    attn     = softmax(scores, -1)
    out      = (attn @ mapped_b) @ Wc + bc

Weights fold per batch (computed on device in exact fp32):
    Wfused    = 8 * Wa @ mapped_b.T               [256, 256]
    scoreBias = 8 * ba @ mapped_b.T               [256]
    Wout      = mapped_b @ Wc                     [256, 256]   (bc added on host)
    scores    = a @ Wfused + scoreBias
    out       = softmax(scores) @ Wout + bc

Numerics: scores run as a 3-term bf16 split (a = ahi+alo on host, Wfused =
Whi+Wlo on device): ahi@Whi + alo@Whi + ahi@Wlo.  Everything downstream of
exp runs in bf16 (attn, Wout, output) which costs ~6e-3 rel err against the
2e-2 budget and makes the PE transposes + final matmuls run at 1 cyc/row
with fast weight loads.

Softmax normalization is deferred to the host: exp() writes *unnormalized*
attn (bf16) and its row-sums z via the scalar-engine accumulator; the device
returns outT' = attnT_unnorm @ Wout (bf16) plus z, and the host computes
out = outT'.T / z + bc.  This removes the per-row normalize multiply and
halves the output DMA (bf16 instead of fp32).

The per-macro loop is software-pipelined: the scores matmuls for macro m+1
are issued before the softmax/transpose/final stage of macro m so the PE
never stalls waiting for the scalar-engine exp.

Layout: input_a transposed on host to [256, seq] (hi/lo bf16 stacked ->
[512, seq]); output produced transposed [256, seq] bf16, transposed back on
host. attn is transposed to [j, i] via PE transposes (bf16, vs identity).
z is accumulated as [128, n_subtiles] (i on partitions) and reshaped on host.
"""
import sys

for _p in ("/opt/trn_rl_repo",):
    if _p not in sys.path:
        sys.path.append(_p)

import numpy as np
import ml_dtypes

import concourse.bacc as bacc
import concourse.mybir as mybir
import concourse.tile as tile
from concourse.bass_utils import run_bass_kernel_spmd

F32 = mybir.dt.float32
BF16 = mybir.dt.bfloat16
I32 = mybir.dt.int32
P = 128

N_CORES = 8
BATCHES_PER_CORE = 2
SEQ = 6209
DF = 256          # feature dim of a / b
HID = 64          # projection dim
DMA_MACRO = 2048  # rows fetched per input DMA instruction
CMACRO = 256      # rows per compute macro (2 subtiles of 128); the scores
                  # psum tile is then a single bank, so bufs=4 fits and the
                  # software pipeline runs two macros deep
NSUB = (SEQ + P - 1) // P  # 49 subtiles per batch (z columns)


def _row_plan(n_rows):
    """[(dma_start, dma_len, [(cm_start_within_dma, cm_len), ...]), ...]

    The first macro is small so compute can start before the bulk of the
    input has landed."""
    plan = []
    pos = 0
    while pos < n_rows:
        d = min(CMACRO if pos == 0 else DMA_MACRO, n_rows - pos)
        cms = []
        q = 0
        while q < d:
            c = min(CMACRO, d - q)
            cms.append((q, c))
            q += c
        plan.append((pos, d, cms))
        pos += d
    return plan


def build_program(seq=SEQ, batches=BATCHES_PER_CORE, use_ba=False):
    nc = bacc.Bacc("TRN2", target_bir_lowering=False, debug=False)

    a_hl = nc.dram_tensor("a_hl", [batches, 2 * DF, seq], BF16, kind="ExternalInput")
    b_t = nc.dram_tensor("b_t", [batches, DF, DF], F32, kind="ExternalInput")
    wat = nc.dram_tensor("wat", [HID, DF], F32, kind="ExternalInput")
    wb = nc.dram_tensor("wb", [DF, HID], F32, kind="ExternalInput")
    wc = nc.dram_tensor("wc", [HID, DF], F32, kind="ExternalInput")
    ba_d = nc.dram_tensor("ba_d", [HID, 1], F32, kind="ExternalInput")
    bb_d = nc.dram_tensor("bb_d", [HID, 1], F32, kind="ExternalInput")
    eye_d = nc.dram_tensor("eye_d", [P, P], BF16, kind="ExternalInput")
    ones_d = nc.dram_tensor("ones_d", [1, P], F32, kind="ExternalInput")
    out_t = nc.dram_tensor("out_t", [batches, DF, seq], BF16, kind="ExternalOutput")
    z_d = nc.dram_tensor("z_d", [batches, P, NSUB], F32, kind="ExternalOutput")

    Exp = mybir.ActivationFunctionType.Exp
    Copy = mybir.ActivationFunctionType.Copy
    Ident = mybir.ActivationFunctionType.Identity

    with tile.TileContext(nc) as tc:
        with (
            tc.tile_pool(name="const", bufs=1) as cpool,
            tc.tile_pool(name="wpool", bufs=2) as wpool,
            tc.tile_pool(name="zpool", bufs=2) as zpool,
            tc.tile_pool(name="apool", bufs=4) as apool,
            tc.tile_pool(name="mpool", bufs=3) as mpool,
            tc.tile_pool(name="opool", bufs=4) as opool,
            tc.tile_pool(name="pp", bufs=1, space="PSUM") as pp,
        ):
            # ---- per-core constants ----
            eye_sb = cpool.tile([P, P], BF16)
            nc.sync.dma_start(eye_sb[:], eye_d[:])
            wat_sb = cpool.tile([HID, DF], F32)
            nc.sync.dma_start(wat_sb[:], wat[:])
            wb_sb = cpool.tile([P, 2, HID], F32)
            nc.sync.dma_start(wb_sb[:], wb[:].rearrange("(k p) h -> p k h", p=P))
            wc_sb = cpool.tile([HID, DF], F32)
            nc.sync.dma_start(wc_sb[:], wc[:])
            ba_sb = cpool.tile([HID, 1], F32)
            nc.sync.dma_start(ba_sb[:], ba_d[:])
            bb_sb = cpool.tile([HID, 1], F32)
            nc.sync.dma_start(bb_sb[:], bb_d[:])
            ones_sb = cpool.tile([1, P], F32)
            nc.sync.dma_start(ones_sb[:], ones_d[:])

            # ---- per-batch fused weights (exact fp32 matmuls) ----
            whis, wlos, wos, sbiases, zsbs = [], [], [], [], []
            for b in range(batches):
                bT_sb = wpool.tile([P, 2, DF], F32, tag="bT")
                nc.sync.dma_start(bT_sb[:], b_t[b].rearrange("(k p) j -> p k j", p=P))

                ps_mb = pp.tile([HID, DF], F32, tag="fin0")
                for k in range(2):
                    nc.tensor.matmul(
                        ps_mb[:],
                        wb_sb[:, k, :],
                        bT_sb[:, k, :],
                        start=(k == 0), stop=(k == 1),
                    )
                mapped_bT = wpool.tile([HID, DF], F32, tag="mbT")
                nc.scalar.activation(mapped_bT[:], ps_mb[:], Ident, bias=bb_sb[:])

                # Wfused, split hi/lo into bf16 (scale 8 folded in)
                whi_sb = wpool.tile([P, 2, DF], BF16, tag="whi")
                wlo_sb = wpool.tile([P, 2, DF], BF16, tag="wlo")
                for c in range(2):
                    ps_wf = pp.tile([P, DF], F32, tag="fin0")
                    nc.tensor.matmul(
                        ps_wf[:],
                        wat_sb[:, c * P:(c + 1) * P],
                        mapped_bT[:],
                        start=True, stop=True,
                    )
                    nc.scalar.activation(whi_sb[:, c, :], ps_wf[:], Copy, scale=8.0)
                    # wlo = 8*wf - whi (rounded to bf16)
                    nc.vector.scalar_tensor_tensor(
                        wlo_sb[:, c, :],
                        ps_wf[:],
                        8.0,
                        whi_sb[:, c, :],
                        op0=mybir.AluOpType.mult,
                        op1=mybir.AluOpType.subtract,
                    )

                sbias_sb = None
                if use_ba:
                    ps_sbias = pp.tile([1, DF], F32, tag="fin1")
                    nc.tensor.matmul(
                        ps_sbias[:],
                        ba_sb[:],
                        mapped_bT[:],
                        start=True, stop=True,
                    )
                    sbias_sb = wpool.tile([1, DF], F32, tag="sbias")
                    nc.scalar.activation(sbias_sb[:], ps_sbias[:], Copy, scale=8.0)

                # Wout[j, f] = mapped_b @ Wc, as lhsT chunks [j(K), f(M)] in bf16
                wo_sb = wpool.tile([P, 2, DF], BF16, tag="wo")
                for c in range(2):
                    ps_wo = pp.tile([P, DF], F32, tag="fin1")
                    nc.tensor.matmul(
                        ps_wo[:],
                        mapped_bT[:, c * P:(c + 1) * P],
                        wc_sb[:],
                        start=True, stop=True,
                    )
                    nc.vector.tensor_copy(wo_sb[:, c, :], ps_wo[:])

                z_sb = zpool.tile([P, NSUB], F32, tag="z")
                whis.append(whi_sb)
                wlos.append(wlo_sb)
                wos.append(wo_sb)
                sbiases.append(sbias_sb)
                zsbs.append(z_sb)

            # ---- software-pipelined main loop over (batch, dma-macro, cmacro) ----
            # stage A (PE): scores matmuls into psum
            # stage B: max -> exp(bf16, accum z) -> transpose -> copies -> final
            def stage_a(b, aT_sb, mo, R, subs):
                whi_sb, wlo_sb = whis[b], wlos[b]
                scores_ps = pp.tile([P, 2 * DF], F32, tag="scores", bufs=4)
                for s, (io, r) in enumerate(subs):
                    c0 = s * DF
                    terms = []
                    for k in range(2):
                        ah = aT_sb[:, k, mo + io:mo + io + r]
                        al = aT_sb[:, 2 + k, mo + io:mo + io + r]
                        # ordered for stationary reuse: ah used twice in a row
                        terms += [
                            (ah, whi_sb[:, k, :]),
                            (ah, wlo_sb[:, k, :]),
                            (al, whi_sb[:, k, :]),
                        ]
                    # accumulate: k=0 triple then k=1 triple
                    terms = terms[0:2] + terms[3:5] + [terms[2], terms[5]]
                    for t, (lhs, rhs) in enumerate(terms):
                        nc.tensor.matmul(
                            scores_ps[:r, c0:c0 + DF],
                            lhs,
                            rhs,
                            start=(t == 0),
                            stop=(t == len(terms) - 1) and not use_ba,
                        )
                    if use_ba:
                        nc.tensor.matmul(
                            scores_ps[:r, c0:c0 + DF],
                            ones_sb[:, :r],
                            sbiases[b][:],
                            start=False, stop=True,
                        )
                return scores_ps

            def stage_b(b, scores_ps, d0, mo, R, subs):
                wo_sb, z_sb = wos[b], zsbs[b]
                rmax = max(r for _, r in subs)
                ns = len(subs)
                uniform = all(r == rmax for _, r in subs)

                negmax = mpool.tile([P, 2], F32, tag="negmax")
                if uniform:
                    nc.vector.tensor_reduce(
                        negmax[:rmax, :ns],
                        scores_ps[:rmax, :ns * DF].rearrange(
                            "p (s j) -> p s j", s=ns),
                        axis=mybir.AxisListType.X,
                        op=mybir.AluOpType.max,
                        negate=True,
                    )
                else:
                    for s, (io, r) in enumerate(subs):
                        nc.vector.tensor_reduce(
                            negmax[:r, s:s + 1],
                            scores_ps[:r, s * DF:(s + 1) * DF],
                            axis=mybir.AxisListType.X,
                            op=mybir.AluOpType.max,
                            negate=True,
                        )

                attn_sb = mpool.tile([P, 2 * DF], BF16, tag="attn")
                aT_ps = pp.tile([P, 2, CMACRO], BF16, tag="attnT", bufs=2)
                t_base = (d0 + mo) // P
                for s, (io, r) in enumerate(subs):
                    c0 = s * DF
                    nc.scalar.activation(
                        attn_sb[:r, c0:c0 + DF],
                        scores_ps[:r, c0:c0 + DF],
                        Exp,
                        bias=negmax[:r, s:s + 1],
                        accum_out=z_sb[:r, t_base + s:t_base + s + 1],
                    )
                # jh-major transposes; each jh half is copied out as soon as
                # its transposes land so the final matmuls start earlier
                attnT = mpool.tile([P, 2, CMACRO], BF16, tag="attnTsb")
                for jh in range(2):
                    for s, (io, r) in enumerate(subs):
                        c0 = s * DF
                        nc.tensor.transpose(
                            aT_ps[:, jh, io:io + r],
                            attn_sb[:r, c0 + jh * P:c0 + (jh + 1) * P],
                            eye_sb[:r, :r],
                        )
                    if R % 2 == 0:
                        nc.vector.tensor_copy(
                            attnT[:, jh, :R].bitcast(I32),
                            aT_ps[:, jh, :R].bitcast(I32))
                    else:
                        nc.vector.tensor_copy(
                            attnT[:, jh, :R], aT_ps[:, jh, :R])

                # final: outT[f, i] = sum_j Wout[j, f] attnT[j, i]
                outT_sb = opool.tile([P, 2, CMACRO], BF16, tag="outT")
                for c in range(2):
                    ps_fin = pp.tile([P, CMACRO], F32, tag=f"fin{c}")
                    for k in range(2):
                        nc.tensor.matmul(
                            ps_fin[:, :R],
                            wos[b][:, k, c * P:(c + 1) * P],
                            attnT[:, k, :R],
                            start=(k == 0), stop=(k == 1),
                        )
                    if c == 0:
                        nc.vector.tensor_copy(
                            outT_sb[:, c, :R], ps_fin[:, :R])
                    else:
                        nc.scalar.copy(
                            outT_sb[:, c, :R], ps_fin[:, :R])
                g0 = d0 + mo
                nc.sync.dma_start(
                    out_t[b][:, g0:g0 + R].rearrange("(c p) i -> p c i", p=P),
                    outT_sb[:, :, :R],
                )

            # build flat list of work items (b, d0, dlen, mo, R, subs)
            items = []
            for b in range(batches):
                for d0, dlen, cms in _row_plan(seq):
                    for mi, (mo, R) in enumerate(cms):
                        subs = [(o, min(P, R - o)) for o in range(0, R, P)]
                        items.append((b, d0, dlen, mo, R, subs,
                                      mi == 0, mi == len(cms) - 1))

            DEPTH = 2
            a_tiles = {}
            pending = []  # [(item, scores_ps), ...]
            for it in items:
                b, d0, dlen, mo, R, subs, first, last = it
                if first:
                    aT_sb = apool.tile([P, 4, DMA_MACRO], BF16, tag="aT")
                    nc.sync.dma_start(
                        aT_sb[:, :, :dlen],
                        a_hl[b][:, d0:d0 + dlen].rearrange(
                            "(k p) i -> p k i", p=P),
                    )
                    a_tiles[(b, d0)] = aT_sb
                sp = stage_a(b, a_tiles[(b, d0)], mo, R, subs)
                pending.append((it, sp))
                if len(pending) > DEPTH:
                    (pb, pd0, _, pmo, pR, psubs, _, _), psp = pending.pop(0)
                    stage_b(pb, psp, pd0, pmo, pR, psubs)
            for (pb, pd0, _, pmo, pR, psubs, _, _), psp in pending:
                stage_b(pb, psp, pd0, pmo, pR, psubs)
            for b in range(batches):
                nc.sync.dma_start(z_d[b], zsbs[b][:])

    nc.compile()
    return nc


_PROGRAM_CACHE = {}


def _get_program(seq=SEQ, batches=BATCHES_PER_CORE, use_ba=False):
    key = (seq, batches, use_ba)
    if key not in _PROGRAM_CACHE:
        _PROGRAM_CACHE[key] = build_program(seq, batches, use_ba)
    return _PROGRAM_CACHE[key]


def make_in_maps(input_a, input_b, Wa, ba, Wb, bb, Wc, bc,
                 n_cores=N_CORES, batches=BATCHES_PER_CORE):
    input_a = np.asarray(input_a, dtype=np.float32)
    input_b = np.asarray(input_b, dtype=np.float32)
    a_t = np.ascontiguousarray(input_a.transpose(0, 2, 1))      # [B, DF, seq]
    a_hi = a_t.astype(ml_dtypes.bfloat16)
    a_lo = (a_t - a_hi.astype(np.float32)).astype(ml_dtypes.bfloat16)
    # rows 0..DF-1 = hi, DF..2DF-1 = lo  -> [B, 2*DF, seq]
    a_hl = np.ascontiguousarray(np.concatenate([a_hi, a_lo], axis=1))
    b_t = np.ascontiguousarray(input_b.transpose(0, 2, 1))
    shared = {
        "wat": np.ascontiguousarray(np.asarray(Wa, np.float32).T),
        "wb": np.ascontiguousarray(np.asarray(Wb, np.float32)),
        "wc": np.ascontiguousarray(np.asarray(Wc, np.float32)),
        "ba_d": np.asarray(ba, np.float32).reshape(HID, 1).copy(),
        "bb_d": np.asarray(bb, np.float32).reshape(HID, 1).copy(),
        "eye_d": np.eye(P, dtype=ml_dtypes.bfloat16),
        "ones_d": np.ones((1, P), dtype=np.float32),
    }
    in_maps = []
    for c in range(n_cores):
        lo, hi = c * batches, (c + 1) * batches
        in_maps.append({
            "a_hl": np.ascontiguousarray(a_hl[lo:hi]),
            "b_t": np.ascontiguousarray(b_t[lo:hi]),
            **shared,
        })
    return in_maps


def postprocess(results, bc, n_cores=N_CORES, batches=BATCHES_PER_CORE):
    """results: list of per-core dicts with out_t [b, DF, seq] bf16 and
    z_d [b, P, NSUB] fp32 -> full [16, seq, DF] fp32 output."""
    bc = np.asarray(bc, np.float32)
    outs = []
    for r in results:
        ot = np.asarray(r["out_t"], np.float32)        # [b, DF, seq]
        z = np.asarray(r["z_d"], np.float32)           # [b, P, NSUB]
        for bi in range(ot.shape[0]):
            zb = z[bi].T.reshape(-1)[:ot.shape[2]]     # [seq]
            outs.append(ot[bi].T / zb[:, None] + bc)
    return np.ascontiguousarray(np.stack(outs, axis=0))


def kernel(input_a, input_b, Wa, ba, Wb, bb, Wc, bc):
    use_ba = bool(np.any(np.asarray(ba)))
    nc = _get_program(use_ba=use_ba)
    in_maps = make_in_maps(input_a, input_b, Wa, ba, Wb, bb, Wc, bc)
    res = run_bass_kernel_spmd(nc, in_maps, core_ids=list(range(N_CORES)))
    return postprocess(res.results, bc)


# revision 22
# speedup vs baseline: 1.0902x; 1.0902x over previous
"""Trainium2 Bass kernel for nn_CrossAttention (16x6209x256 cross-attention).

Strategy
--------
Data-parallel over batch: 16 batches -> 8 cores x 2 batches, pure SPMD.

Per batch:
    mapped_a = a @ Wa + ba            [6209, 64]
    mapped_b = b @ Wb + bb            [256, 64]
    scores   = mapped_a @ mapped_b.T * 8
    attn     = softmax(scores, -1)
    out      = (attn @ mapped_b) @ Wc + bc

Weights fold per batch (computed on device in exact fp32):
    Wfused    = 8 * Wa @ mapped_b.T               [256, 256]
    scoreBias = 8 * ba @ mapped_b.T               [256]
    Wout      = mapped_b @ Wc                     [256, 256]   (bc added on host)
    scores    = a @ Wfused + scoreBias
    out       = softmax(scores) @ Wout + bc

Numerics: scores run as a 3-term bf16 split (a = ahi+alo on host, Wfused =
Whi+Wlo on device): ahi@Whi + alo@Whi + ahi@Wlo.  Everything downstream of
exp runs in bf16 (attn, Wout, output) which costs ~6e-3 rel err against the
2e-2 budget and makes the PE transposes + final matmuls run at 1 cyc/row
with fast weight loads.

Softmax normalization is deferred to the host: exp() writes *unnormalized*
attn (bf16) and its row-sums z via the scalar-engine accumulator; the device
returns outT' = attnT_unnorm @ Wout (bf16) plus z, and the host computes
out = outT'.T / z + bc.  This removes the per-row normalize multiply and
halves the output DMA (bf16 instead of fp32).

The per-macro loop is software-pipelined: the scores matmuls for macro m+1
are issued before the softmax/transpose/final stage of macro m so the PE
never stalls waiting for the scalar-engine exp.

Layout: input_a transposed on host to [256, seq] (hi/lo bf16 stacked ->
[512, seq]); output produced transposed [256, seq] bf16, transposed back on
host. attn is transposed to [j, i] via PE transposes (bf16, vs identity).
z is accumulated as [128, n_subtiles] (i on partitions) and reshaped on host.
"""
import sys

for _p in ("/opt/trn_rl_repo",):
    if _p not in sys.path:
        sys.path.append(_p)

import numpy as np
import ml_dtypes

import concourse.bacc as bacc
import concourse.mybir as mybir
import concourse.tile as tile
from concourse.bass_utils import run_bass_kernel_spmd

F32 = mybir.dt.float32
BF16 = mybir.dt.bfloat16
I32 = mybir.dt.int32
P = 128

N_CORES = 8
BATCHES_PER_CORE = 2
SEQ = 6209
DF = 256          # feature dim of a / b
HID = 64          # projection dim
DMA_MACRO = 2048  # rows fetched/stored per DMA instruction
CMACRO = 512      # rows per compute macro (4 subtiles of 128)
NSUB = (SEQ + P - 1) // P  # 49 subtiles per batch (z columns)


def _row_plan(n_rows):
    """[(dma_start, dma_len, [(cm_start_within_dma, cm_len), ...]), ...]

    The first macro is small so compute can start before the bulk of the
    input has landed."""
    plan = []
    pos = 0
    while pos < n_rows:
        d = min(CMACRO if pos == 0 else DMA_MACRO, n_rows - pos)
        cms = []
        q = 0
        while q < d:
            c = min(CMACRO, d - q)
            cms.append((q, c))
            q += c
        plan.append((pos, d, cms))
        pos += d
    return plan


def build_program(seq=SEQ, batches=BATCHES_PER_CORE, use_ba=False):
    nc = bacc.Bacc("TRN2", target_bir_lowering=False, debug=False)

    a_hl = nc.dram_tensor("a_hl", [batches, 2 * DF, seq], BF16, kind="ExternalInput")
    b_t = nc.dram_tensor("b_t", [batches, DF, DF], F32, kind="ExternalInput")
    wat = nc.dram_tensor("wat", [HID, DF], F32, kind="ExternalInput")
    wb = nc.dram_tensor("wb", [DF, HID], F32, kind="ExternalInput")
    wc = nc.dram_tensor("wc", [HID, DF], F32, kind="ExternalInput")
    ba_d = nc.dram_tensor("ba_d", [HID, 1], F32, kind="ExternalInput")
    bb_d = nc.dram_tensor("bb_d", [HID, 1], F32, kind="ExternalInput")
    eye_d = nc.dram_tensor("eye_d", [P, P], BF16, kind="ExternalInput")
    ones_d = nc.dram_tensor("ones_d", [1, P], F32, kind="ExternalInput")
    out_t = nc.dram_tensor("out_t", [batches, DF, seq], BF16, kind="ExternalOutput")
    z_d = nc.dram_tensor("z_d", [batches, P, NSUB], F32, kind="ExternalOutput")

    Exp = mybir.ActivationFunctionType.Exp
    Copy = mybir.ActivationFunctionType.Copy
    Ident = mybir.ActivationFunctionType.Identity

    with tile.TileContext(nc) as tc:
        with (
            tc.tile_pool(name="const", bufs=1) as cpool,
            tc.tile_pool(name="wpool", bufs=2) as wpool,
            tc.tile_pool(name="zpool", bufs=2) as zpool,
            tc.tile_pool(name="apool", bufs=4) as apool,
            tc.tile_pool(name="mpool", bufs=3) as mpool,
            tc.tile_pool(name="opool", bufs=4) as opool,
            tc.tile_pool(name="pp", bufs=1, space="PSUM") as pp,
        ):
            # ---- per-core constants ----
            eye_sb = cpool.tile([P, P], BF16)
            nc.sync.dma_start(eye_sb[:], eye_d[:])
            wat_sb = cpool.tile([HID, DF], F32)
            nc.sync.dma_start(wat_sb[:], wat[:])
            wb_sb = cpool.tile([P, 2, HID], F32)
            nc.sync.dma_start(wb_sb[:], wb[:].rearrange("(k p) h -> p k h", p=P))
            wc_sb = cpool.tile([HID, DF], F32)
            nc.sync.dma_start(wc_sb[:], wc[:])
            ba_sb = cpool.tile([HID, 1], F32)
            nc.sync.dma_start(ba_sb[:], ba_d[:])
            bb_sb = cpool.tile([HID, 1], F32)
            nc.sync.dma_start(bb_sb[:], bb_d[:])
            ones_sb = cpool.tile([1, P], F32)
            nc.sync.dma_start(ones_sb[:], ones_d[:])

            # ---- per-batch fused weights (exact fp32 matmuls) ----
            whis, wlos, wos, sbiases, zsbs = [], [], [], [], []
            for b in range(batches):
                bT_sb = wpool.tile([P, 2, DF], F32, tag="bT")
                nc.sync.dma_start(bT_sb[:], b_t[b].rearrange("(k p) j -> p k j", p=P))

                ps_mb = pp.tile([HID, DF], F32, tag="fin0")
                for k in range(2):
                    nc.tensor.matmul(
                        ps_mb[:],
                        wb_sb[:, k, :],
                        bT_sb[:, k, :],
                        start=(k == 0), stop=(k == 1),
                    )
                mapped_bT = wpool.tile([HID, DF], F32, tag="mbT")
                nc.scalar.activation(mapped_bT[:], ps_mb[:], Ident, bias=bb_sb[:])

                # Wfused, split hi/lo into bf16 (scale 8 folded in)
                whi_sb = wpool.tile([P, 2, DF], BF16, tag="whi")
                wlo_sb = wpool.tile([P, 2, DF], BF16, tag="wlo")
                for c in range(2):
                    ps_wf = pp.tile([P, DF], F32, tag="fin0")
                    nc.tensor.matmul(
                        ps_wf[:],
                        wat_sb[:, c * P:(c + 1) * P],
                        mapped_bT[:],
                        start=True, stop=True,
                    )
                    nc.scalar.activation(whi_sb[:, c, :], ps_wf[:], Copy, scale=8.0)
                    # wlo = 8*wf - whi (rounded to bf16)
                    nc.vector.scalar_tensor_tensor(
                        wlo_sb[:, c, :],
                        ps_wf[:],
                        8.0,
                        whi_sb[:, c, :],
                        op0=mybir.AluOpType.mult,
                        op1=mybir.AluOpType.subtract,
                    )

                sbias_sb = None
                if use_ba:
                    ps_sbias = pp.tile([1, DF], F32, tag="fin1")
                    nc.tensor.matmul(
                        ps_sbias[:],
                        ba_sb[:],
                        mapped_bT[:],
                        start=True, stop=True,
                    )
                    sbias_sb = wpool.tile([1, DF], F32, tag="sbias")
                    nc.scalar.activation(sbias_sb[:], ps_sbias[:], Copy, scale=8.0)

                # Wout[j, f] = mapped_b @ Wc, as lhsT chunks [j(K), f(M)] in bf16
                wo_sb = wpool.tile([P, 2, DF], BF16, tag="wo")
                for c in range(2):
                    ps_wo = pp.tile([P, DF], F32, tag="fin1")
                    nc.tensor.matmul(
                        ps_wo[:],
                        mapped_bT[:, c * P:(c + 1) * P],
                        wc_sb[:],
                        start=True, stop=True,
                    )
                    nc.vector.tensor_copy(wo_sb[:, c, :], ps_wo[:])

                z_sb = zpool.tile([P, NSUB], F32, tag="z")
                whis.append(whi_sb)
                wlos.append(wlo_sb)
                wos.append(wo_sb)
                sbiases.append(sbias_sb)
                zsbs.append(z_sb)

            # ---- software-pipelined main loop over (batch, dma-macro, cmacro) ----
            # stage A (PE): scores matmuls into psum
            # stage B: max -> exp(bf16, accum z) -> transpose -> copies -> final
            def stage_a(b, aT_sb, mo, R, subs):
                whi_sb, wlo_sb = whis[b], wlos[b]
                scores_ps = pp.tile([P, 4 * DF], F32, tag="scores", bufs=2)
                for s, (io, r) in enumerate(subs):
                    c0 = s * DF
                    terms = []
                    for k in range(2):
                        ah = aT_sb[:, k, mo + io:mo + io + r]
                        al = aT_sb[:, 2 + k, mo + io:mo + io + r]
                        # ordered for stationary reuse: ah used twice in a row
                        terms += [
                            (ah, whi_sb[:, k, :]),
                            (ah, wlo_sb[:, k, :]),
                            (al, whi_sb[:, k, :]),
                        ]
                    # accumulate: k=0 triple then k=1 triple
                    terms = terms[0:2] + terms[3:5] + [terms[2], terms[5]]
                    for t, (lhs, rhs) in enumerate(terms):
                        nc.tensor.matmul(
                            scores_ps[:r, c0:c0 + DF],
                            lhs,
                            rhs,
                            start=(t == 0),
                            stop=(t == len(terms) - 1) and not use_ba,
                        )
                    if use_ba:
                        nc.tensor.matmul(
                            scores_ps[:r, c0:c0 + DF],
                            ones_sb[:, :r],
                            sbiases[b][:],
                            start=False, stop=True,
                        )
                return scores_ps

            def stage_b(b, scores_ps, d0, mo, R, subs):
                wo_sb, z_sb = wos[b], zsbs[b]
                rmax = max(r for _, r in subs)
                ns = len(subs)
                uniform = all(r == rmax for _, r in subs)

                negmax = mpool.tile([P, 4], F32, tag="negmax")
                if uniform:
                    nc.vector.tensor_reduce(
                        negmax[:rmax, :ns],
                        scores_ps[:rmax, :ns * DF].rearrange(
                            "p (s j) -> p s j", s=ns),
                        axis=mybir.AxisListType.X,
                        op=mybir.AluOpType.max,
                        negate=True,
                    )
                else:
                    for s, (io, r) in enumerate(subs):
                        nc.vector.tensor_reduce(
                            negmax[:r, s:s + 1],
                            scores_ps[:r, s * DF:(s + 1) * DF],
                            axis=mybir.AxisListType.X,
                            op=mybir.AluOpType.max,
                            negate=True,
                        )

                attn_sb = mpool.tile([P, 4 * DF], BF16, tag="attn")
                aT_ps = pp.tile([P, 2, CMACRO], BF16, tag="attnT", bufs=2)
                t_base = (d0 + mo) // P
                for s, (io, r) in enumerate(subs):
                    c0 = s * DF
                    nc.scalar.activation(
                        attn_sb[:r, c0:c0 + DF],
                        scores_ps[:r, c0:c0 + DF],
                        Exp,
                        bias=negmax[:r, s:s + 1],
                        accum_out=z_sb[:r, t_base + s:t_base + s + 1],
                    )
                # jh-major transposes: each jh half is copied out as soon as
                # its transposes land so the final matmuls start earlier
                attnT = mpool.tile([P, 2, CMACRO], BF16, tag="attnTsb")
                for jh in range(2):
                    for s, (io, r) in enumerate(subs):
                        c0 = s * DF
                        nc.tensor.transpose(
                            aT_ps[:, jh, io:io + r],
                            attn_sb[:r, c0 + jh * P:c0 + (jh + 1) * P],
                            eye_sb[:r, :r],
                        )
                    if R % 2 == 0:
                        nc.vector.tensor_copy(
                            attnT[:, jh, :R].bitcast(I32),
                            aT_ps[:, jh, :R].bitcast(I32))
                    else:
                        nc.vector.tensor_copy(
                            attnT[:, jh, :R], aT_ps[:, jh, :R])

                # final: outT[f, i] = sum_j Wout[j, f] attnT[j, i]
                outT_sb = opool.tile([P, 2, CMACRO], BF16, tag="outT")
                for c in range(2):
                    ps_fin = pp.tile([P, CMACRO], F32, tag=f"fin{c}")
                    for k in range(2):
                        nc.tensor.matmul(
                            ps_fin[:, :R],
                            wos[b][:, k, c * P:(c + 1) * P],
                            attnT[:, k, :R],
                            start=(k == 0), stop=(k == 1),
                        )
                    if c == 0:
                        nc.vector.tensor_copy(
                            outT_sb[:, c, :R], ps_fin[:, :R])
                    else:
                        nc.scalar.copy(
                            outT_sb[:, c, :R], ps_fin[:, :R])
                g0 = d0 + mo
                nc.sync.dma_start(
                    out_t[b][:, g0:g0 + R].rearrange("(c p) i -> p c i", p=P),
                    outT_sb[:, :, :R],
                )

            # build flat list of work items (b, d0, dlen, mo, R, subs)
            items = []
            for b in range(batches):
                for d0, dlen, cms in _row_plan(seq):
                    for mi, (mo, R) in enumerate(cms):
                        subs = [(o, min(P, R - o)) for o in range(0, R, P)]
                        items.append((b, d0, dlen, mo, R, subs,
                                      mi == 0, mi == len(cms) - 1))

            a_tiles = {}
            pending = None  # (item, scores_ps)
            for it in items:
                b, d0, dlen, mo, R, subs, first, last = it
                if first:
                    aT_sb = apool.tile([P, 4, DMA_MACRO], BF16, tag="aT")
                    nc.sync.dma_start(
                        aT_sb[:, :, :dlen],
                        a_hl[b][:, d0:d0 + dlen].rearrange(
                            "(k p) i -> p k i", p=P),
                    )
                    a_tiles[(b, d0)] = aT_sb
                sp = stage_a(b, a_tiles[(b, d0)], mo, R, subs)
                if pending is not None:
                    pb, pd0, pdlen, pmo, pR, psubs, _, _ = pending[0]
                    stage_b(pb, pending[1], pd0, pmo, pR, psubs)
                pending = (it, sp)
            # drain
            pb, pd0, pdlen, pmo, pR, psubs, _, _ = pending[0]
            stage_b(pb, pending[1], pd0, pmo, pR, psubs)
            for b in range(batches):
                nc.sync.dma_start(z_d[b], zsbs[b][:])

    nc.compile()
    return nc


_PROGRAM_CACHE = {}


def _get_program(seq=SEQ, batches=BATCHES_PER_CORE, use_ba=False):
    key = (seq, batches, use_ba)
    if key not in _PROGRAM_CACHE:
        _PROGRAM_CACHE[key] = build_program(seq, batches, use_ba)
    return _PROGRAM_CACHE[key]


def make_in_maps(input_a, input_b, Wa, ba, Wb, bb, Wc, bc,
                 n_cores=N_CORES, batches=BATCHES_PER_CORE):
    input_a = np.asarray(input_a, dtype=np.float32)
    input_b = np.asarray(input_b, dtype=np.float32)
    a_t = np.ascontiguousarray(input_a.transpose(0, 2, 1))      # [B, DF, seq]
    a_hi = a_t.astype(ml_dtypes.bfloat16)
    a_lo = (a_t - a_hi.astype(np.float32)).astype(ml_dtypes.bfloat16)
    # rows 0..DF-1 = hi, DF..2DF-1 = lo  -> [B, 2*DF, seq]
    a_hl = np.ascontiguousarray(np.concatenate([a_hi, a_lo], axis=1))
    b_t = np.ascontiguousarray(input_b.transpose(0, 2, 1))
    shared = {
        "wat": np.ascontiguousarray(np.asarray(Wa, np.float32).T),
        "wb": np.ascontiguousarray(np.asarray(Wb, np.float32)),
        "wc": np.ascontiguousarray(np.asarray(Wc, np.float32)),
        "ba_d": np.asarray(ba, np.float32).reshape(HID, 1).copy(),
        "bb_d": np.asarray(bb, np.float32).reshape(HID, 1).copy(),
        "eye_d": np.eye(P, dtype=ml_dtypes.bfloat16),
        "ones_d": np.ones((1, P), dtype=np.float32),
    }
    in_maps = []
    for c in range(n_cores):
        lo, hi = c * batches, (c + 1) * batches
        in_maps.append({
            "a_hl": np.ascontiguousarray(a_hl[lo:hi]),
            "b_t": np.ascontiguousarray(b_t[lo:hi]),
            **shared,
        })
    return in_maps


def postprocess(results, bc, n_cores=N_CORES, batches=BATCHES_PER_CORE):
    """results: list of per-core dicts with out_t [b, DF, seq] bf16 and
    z_d [b, P, NSUB] fp32 -> full [16, seq, DF] fp32 output."""
    bc = np.asarray(bc, np.float32)
    outs = []
    for r in results:
        ot = np.asarray(r["out_t"], np.float32)        # [b, DF, seq]
        z = np.asarray(r["z_d"], np.float32)           # [b, P, NSUB]
        for bi in range(ot.shape[0]):
            zb = z[bi].T.reshape(-1)[:ot.shape[2]]     # [seq]
            outs.append(ot[bi].T / zb[:, None] + bc)
    return np.ascontiguousarray(np.stack(outs, axis=0))


def kernel(input_a, input_b, Wa, ba, Wb, bb, Wc, bc):
    use_ba = bool(np.any(np.asarray(ba)))
    nc = _get_program(use_ba=use_ba)
    in_maps = make_in_maps(input_a, input_b, Wa, ba, Wb, bb, Wc, bc)
    res = run_bass_kernel_spmd(nc, in_maps, core_ids=list(range(N_CORES)))
    return postprocess(res.results, bc)


# revision 24
# speedup vs baseline: 1.1112x; 1.0193x over previous
"""Trainium2 Bass kernel for nn_CrossAttention (16x6209x256 cross-attention).

Strategy
--------
Data-parallel over batch: 16 batches -> 8 cores x 2 batches, pure SPMD.

Per batch:
    mapped_a = a @ Wa + ba            [6209, 64]
    mapped_b = b @ Wb + bb            [256, 64]
    scores   = mapped_a @ mapped_b.T * 8
    attn     = softmax(scores, -1)
    out      = (attn @ mapped_b) @ Wc + bc

Weights fold per batch (computed on device in exact fp32):
    Wfused    = 8 * Wa @ mapped_b.T               [256, 256]
    scoreBias = 8 * ba @ mapped_b.T               [256]
    Wout      = mapped_b @ Wc                     [256, 256]   (bc added on host)
    scores    = a @ Wfused + scoreBias
    out       = softmax(scores) @ Wout + bc

Numerics: scores run as a 3-term bf16 split (a = ahi+alo on host, Wfused =
Whi+Wlo on device): ahi@Whi + alo@Whi + ahi@Wlo.  Everything downstream of
exp runs in bf16 (attn, Wout, output) which costs ~6e-3 rel err against the
2e-2 budget and makes the PE transposes + final matmuls run at 1 cyc/row
with fast weight loads.

Softmax normalization is deferred to the host: exp() writes *unnormalized*
attn (bf16) and its row-sums z via the scalar-engine accumulator; the device
returns outT' = attnT_unnorm @ Wout (bf16) plus z, and the host computes
out = outT'.T / z + bc.  This removes the per-row normalize multiply and
halves the output DMA (bf16 instead of fp32).

The per-macro loop is software-pipelined: the scores matmuls for macro m+1
are issued before the softmax/transpose/final stage of macro m so the PE
never stalls waiting for the scalar-engine exp.

Layout: input_a transposed on host to [256, seq] (hi/lo bf16 stacked ->
[512, seq]); output produced transposed [256, seq] bf16, transposed back on
host. attn is transposed to [j, i] via PE transposes (bf16, vs identity).
z is accumulated as [128, n_subtiles] (i on partitions) and reshaped on host.
"""
import sys

for _p in ("/opt/trn_rl_repo",):
    if _p not in sys.path:
        sys.path.append(_p)

import numpy as np
import ml_dtypes

import concourse.bacc as bacc
import concourse.mybir as mybir
import concourse.tile as tile
from concourse.bass_utils import run_bass_kernel_spmd

F32 = mybir.dt.float32
BF16 = mybir.dt.bfloat16
I32 = mybir.dt.int32
P = 128

N_CORES = 8
BATCHES_PER_CORE = 2
SEQ = 6209
DF = 256          # feature dim of a / b
HID = 64          # projection dim
DMA_MACRO = 2048  # rows fetched/stored per DMA instruction
CMACRO = 512      # rows per compute macro (4 subtiles of 128)
NSUB = (SEQ + P - 1) // P  # 49 subtiles per batch (z columns)


def _row_plan(n_rows):
    """[(dma_start, dma_len, [(cm_start_within_dma, cm_len), ...]), ...]

    The first macro is small so compute can start before the bulk of the
    input has landed."""
    plan = []
    pos = 0
    while pos < n_rows:
        d = min(CMACRO if pos == 0 else DMA_MACRO, n_rows - pos)
        cms = []
        q = 0
        while q < d:
            c = min(CMACRO, d - q)
            cms.append((q, c))
            q += c
        plan.append((pos, d, cms))
        pos += d
    return plan


def build_program(seq=SEQ, batches=BATCHES_PER_CORE, use_ba=False):
    nc = bacc.Bacc("TRN2", target_bir_lowering=False, debug=False)

    a_hl = nc.dram_tensor("a_hl", [batches, 2 * DF, seq], BF16, kind="ExternalInput")
    b_t = nc.dram_tensor("b_t", [batches, DF, DF], F32, kind="ExternalInput")
    wat = nc.dram_tensor("wat", [HID, DF], F32, kind="ExternalInput")
    wb = nc.dram_tensor("wb", [DF, HID], F32, kind="ExternalInput")
    wc = nc.dram_tensor("wc", [HID, DF], F32, kind="ExternalInput")
    ba_d = nc.dram_tensor("ba_d", [HID, 1], F32, kind="ExternalInput")
    bb_d = nc.dram_tensor("bb_d", [HID, 1], F32, kind="ExternalInput")
    eye_d = nc.dram_tensor("eye_d", [P, P], BF16, kind="ExternalInput")
    ones_d = nc.dram_tensor("ones_d", [1, P], F32, kind="ExternalInput")
    out_t = nc.dram_tensor("out_t", [batches, DF, seq], BF16, kind="ExternalOutput")
    z_d = nc.dram_tensor("z_d", [batches, P, NSUB], F32, kind="ExternalOutput")

    Exp = mybir.ActivationFunctionType.Exp
    Copy = mybir.ActivationFunctionType.Copy
    Ident = mybir.ActivationFunctionType.Identity

    with tile.TileContext(nc) as tc:
        with (
            tc.tile_pool(name="const", bufs=1) as cpool,
            tc.tile_pool(name="wpool", bufs=2) as wpool,
            tc.tile_pool(name="zpool", bufs=2) as zpool,
            tc.tile_pool(name="apool", bufs=4) as apool,
            tc.tile_pool(name="mpool", bufs=3) as mpool,
            tc.tile_pool(name="opool", bufs=4) as opool,
            tc.tile_pool(name="pp", bufs=1, space="PSUM") as pp,
        ):
            # ---- per-core constants ----
            eye_sb = cpool.tile([P, P], BF16)
            nc.sync.dma_start(eye_sb[:], eye_d[:])
            wat_sb = cpool.tile([HID, DF], F32)
            nc.sync.dma_start(wat_sb[:], wat[:])
            wb_sb = cpool.tile([P, 2, HID], F32)
            nc.sync.dma_start(wb_sb[:], wb[:].rearrange("(k p) h -> p k h", p=P))
            wc_sb = cpool.tile([HID, DF], F32)
            nc.sync.dma_start(wc_sb[:], wc[:])
            ba_sb = cpool.tile([HID, 1], F32)
            nc.sync.dma_start(ba_sb[:], ba_d[:])
            bb_sb = cpool.tile([HID, 1], F32)
            nc.sync.dma_start(bb_sb[:], bb_d[:])
            ones_sb = cpool.tile([1, P], F32)
            nc.sync.dma_start(ones_sb[:], ones_d[:])

            # ---- per-batch fused weights (exact fp32 matmuls) ----
            whis, wlos, wos, sbiases, zsbs = [], [], [], [], []
            for b in range(batches):
                bT_sb = wpool.tile([P, 2, DF], F32, tag="bT")
                nc.sync.dma_start(bT_sb[:], b_t[b].rearrange("(k p) j -> p k j", p=P))

                ps_mb = pp.tile([HID, DF], F32, tag="fin0")
                for k in range(2):
                    nc.tensor.matmul(
                        ps_mb[:],
                        wb_sb[:, k, :],
                        bT_sb[:, k, :],
                        start=(k == 0), stop=(k == 1),
                    )
                mapped_bT = wpool.tile([HID, DF], F32, tag="mbT")
                nc.scalar.activation(mapped_bT[:], ps_mb[:], Ident, bias=bb_sb[:])

                # Wfused, split hi/lo into bf16 (scale 8 folded in)
                whi_sb = wpool.tile([P, 2, DF], BF16, tag="whi")
                wlo_sb = wpool.tile([P, 2, DF], BF16, tag="wlo")
                for c in range(2):
                    ps_wf = pp.tile([P, DF], F32, tag="fin0")
                    nc.tensor.matmul(
                        ps_wf[:],
                        wat_sb[:, c * P:(c + 1) * P],
                        mapped_bT[:],
                        start=True, stop=True,
                    )
                    nc.scalar.activation(whi_sb[:, c, :], ps_wf[:], Copy, scale=8.0)
                    # wlo = 8*wf - whi (rounded to bf16)
                    nc.vector.scalar_tensor_tensor(
                        wlo_sb[:, c, :],
                        ps_wf[:],
                        8.0,
                        whi_sb[:, c, :],
                        op0=mybir.AluOpType.mult,
                        op1=mybir.AluOpType.subtract,
                    )

                sbias_sb = None
                if use_ba:
                    ps_sbias = pp.tile([1, DF], F32, tag="fin1")
                    nc.tensor.matmul(
                        ps_sbias[:],
                        ba_sb[:],
                        mapped_bT[:],
                        start=True, stop=True,
                    )
                    sbias_sb = wpool.tile([1, DF], F32, tag="sbias")
                    nc.scalar.activation(sbias_sb[:], ps_sbias[:], Copy, scale=8.0)

                # Wout[j, f] = mapped_b @ Wc, as lhsT chunks [j(K), f(M)] in bf16
                wo_sb = wpool.tile([P, 2, DF], BF16, tag="wo")
                for c in range(2):
                    ps_wo = pp.tile([P, DF], F32, tag="fin1")
                    nc.tensor.matmul(
                        ps_wo[:],
                        mapped_bT[:, c * P:(c + 1) * P],
                        wc_sb[:],
                        start=True, stop=True,
                    )
                    nc.vector.tensor_copy(wo_sb[:, c, :], ps_wo[:])

                z_sb = zpool.tile([P, NSUB], F32, tag="z")
                whis.append(whi_sb)
                wlos.append(wlo_sb)
                wos.append(wo_sb)
                sbiases.append(sbias_sb)
                zsbs.append(z_sb)

            # ---- software-pipelined main loop over (batch, dma-macro, cmacro) ----
            # stage A (PE): scores matmuls into psum
            # stage B: max -> exp(bf16, accum z) -> transpose -> copies -> final
            def stage_a(b, aT_sb, mo, R, subs):
                whi_sb, wlo_sb = whis[b], wlos[b]
                scores_ps = pp.tile([P, 4 * DF], F32, tag="scores", bufs=2)
                for s, (io, r) in enumerate(subs):
                    c0 = s * DF
                    terms = []
                    for k in range(2):
                        ah = aT_sb[:, k, mo + io:mo + io + r]
                        al = aT_sb[:, 2 + k, mo + io:mo + io + r]
                        # ordered for stationary reuse: ah used twice in a row
                        terms += [
                            (ah, whi_sb[:, k, :]),
                            (ah, wlo_sb[:, k, :]),
                            (al, whi_sb[:, k, :]),
                        ]
                    # accumulate: k=0 triple then k=1 triple
                    terms = terms[0:2] + terms[3:5] + [terms[2], terms[5]]
                    for t, (lhs, rhs) in enumerate(terms):
                        nc.tensor.matmul(
                            scores_ps[:r, c0:c0 + DF],
                            lhs,
                            rhs,
                            start=(t == 0),
                            stop=(t == len(terms) - 1) and not use_ba,
                        )
                    if use_ba:
                        nc.tensor.matmul(
                            scores_ps[:r, c0:c0 + DF],
                            ones_sb[:, :r],
                            sbiases[b][:],
                            start=False, stop=True,
                        )
                return scores_ps

            def stage_b(b, scores_ps, d0, mo, R, subs):
                wo_sb, z_sb = wos[b], zsbs[b]
                rmax = max(r for _, r in subs)
                ns = len(subs)
                uniform = all(r == rmax for _, r in subs)

                negmax = mpool.tile([P, 4], F32, tag="negmax")
                if uniform:
                    nc.vector.tensor_reduce(
                        negmax[:rmax, :ns],
                        scores_ps[:rmax, :ns * DF].rearrange(
                            "p (s j) -> p s j", s=ns),
                        axis=mybir.AxisListType.X,
                        op=mybir.AluOpType.max,
                        negate=True,
                    )
                else:
                    for s, (io, r) in enumerate(subs):
                        nc.vector.tensor_reduce(
                            negmax[:r, s:s + 1],
                            scores_ps[:r, s * DF:(s + 1) * DF],
                            axis=mybir.AxisListType.X,
                            op=mybir.AluOpType.max,
                            negate=True,
                        )

                attn_sb = mpool.tile([P, 4 * DF], BF16, tag="attn")
                aT_ps = pp.tile([P, 2, CMACRO], BF16, tag="attnT", bufs=2)
                t_base = (d0 + mo) // P
                for s, (io, r) in enumerate(subs):
                    c0 = s * DF
                    nc.scalar.activation(
                        attn_sb[:r, c0:c0 + DF],
                        scores_ps[:r, c0:c0 + DF],
                        Exp,
                        bias=negmax[:r, s:s + 1],
                        accum_out=z_sb[:r, t_base + s:t_base + s + 1],
                    )
                    for jh in range(2):
                        nc.tensor.transpose(
                            aT_ps[:, jh, io:io + r],
                            attn_sb[:r, c0 + jh * P:c0 + (jh + 1) * P],
                            eye_sb[:r, :r],
                        )
                attnT = mpool.tile([P, 2, CMACRO], BF16, tag="attnTsb")
                if R % 2 == 0:
                    nc.vector.tensor_copy(
                        attnT[:, :, :R].bitcast(I32), aT_ps[:, :, :R].bitcast(I32))
                else:
                    nc.vector.tensor_copy(attnT[:, 0, :R], aT_ps[:, 0, :R])
                    nc.vector.tensor_copy(attnT[:, 1, :R], aT_ps[:, 1, :R])

                # final: outT[f, i] = sum_j Wout[j, f] attnT[j, i]
                outT_sb = opool.tile([P, 2, CMACRO], BF16, tag="outT")
                for c in range(2):
                    ps_fin = pp.tile([P, CMACRO], F32, tag=f"fin{c}")
                    for k in range(2):
                        nc.tensor.matmul(
                            ps_fin[:, :R],
                            wos[b][:, k, c * P:(c + 1) * P],
                            attnT[:, k, :R],
                            start=(k == 0), stop=(k == 1),
                        )
                    if c == 0:
                        nc.vector.tensor_copy(
                            outT_sb[:, c, :R], ps_fin[:, :R])
                    else:
                        nc.scalar.copy(
                            outT_sb[:, c, :R], ps_fin[:, :R])
                g0 = d0 + mo
                nc.sync.dma_start(
                    out_t[b][:, g0:g0 + R].rearrange("(c p) i -> p c i", p=P),
                    outT_sb[:, :, :R],
                )

            # build flat list of work items (b, d0, dlen, mo, R, subs)
            items = []
            for b in range(batches):
                for d0, dlen, cms in _row_plan(seq):
                    for mi, (mo, R) in enumerate(cms):
                        subs = [(o, min(P, R - o)) for o in range(0, R, P)]
                        items.append((b, d0, dlen, mo, R, subs,
                                      mi == 0, mi == len(cms) - 1))

            a_tiles = {}
            pending = None  # (item, scores_ps)
            for it in items:
                b, d0, dlen, mo, R, subs, first, last = it
                if first:
                    aT_sb = apool.tile([P, 4, DMA_MACRO], BF16, tag="aT")
                    nc.sync.dma_start(
                        aT_sb[:, :, :dlen],
                        a_hl[b][:, d0:d0 + dlen].rearrange(
                            "(k p) i -> p k i", p=P),
                    )
                    a_tiles[(b, d0)] = aT_sb
                sp = stage_a(b, a_tiles[(b, d0)], mo, R, subs)
                if pending is not None:
                    pb, pd0, pdlen, pmo, pR, psubs, _, _ = pending[0]
                    stage_b(pb, pending[1], pd0, pmo, pR, psubs)
                    if pb != b:
                        # batch pb's z is complete; DMA it out now so it
                        # overlaps the next batch's compute
                        nc.sync.dma_start(z_d[pb], zsbs[pb][:])
                pending = (it, sp)
            # drain
            pb, pd0, pdlen, pmo, pR, psubs, _, _ = pending[0]
            stage_b(pb, pending[1], pd0, pmo, pR, psubs)
            nc.sync.dma_start(z_d[pb], zsbs[pb][:])

    nc.compile()
    return nc


_PROGRAM_CACHE = {}


def _get_program(seq=SEQ, batches=BATCHES_PER_CORE, use_ba=False):
    key = (seq, batches, use_ba)
    if key not in _PROGRAM_CACHE:
        _PROGRAM_CACHE[key] = build_program(seq, batches, use_ba)
    return _PROGRAM_CACHE[key]


def make_in_maps(input_a, input_b, Wa, ba, Wb, bb, Wc, bc,
                 n_cores=N_CORES, batches=BATCHES_PER_CORE):
    input_a = np.asarray(input_a, dtype=np.float32)
    input_b = np.asarray(input_b, dtype=np.float32)
    a_t = np.ascontiguousarray(input_a.transpose(0, 2, 1))      # [B, DF, seq]
    a_hi = a_t.astype(ml_dtypes.bfloat16)
    a_lo = (a_t - a_hi.astype(np.float32)).astype(ml_dtypes.bfloat16)
    # rows 0..DF-1 = hi, DF..2DF-1 = lo  -> [B, 2*DF, seq]
    a_hl = np.ascontiguousarray(np.concatenate([a_hi, a_lo], axis=1))
    b_t = np.ascontiguousarray(input_b.transpose(0, 2, 1))
    shared = {
        "wat": np.ascontiguousarray(np.asarray(Wa, np.float32).T),
        "wb": np.ascontiguousarray(np.asarray(Wb, np.float32)),
        "wc": np.ascontiguousarray(np.asarray(Wc, np.float32)),
        "ba_d": np.asarray(ba, np.float32).reshape(HID, 1).copy(),
        "bb_d": np.asarray(bb, np.float32).reshape(HID, 1).copy(),
        "eye_d": np.eye(P, dtype=ml_dtypes.bfloat16),
        "ones_d": np.ones((1, P), dtype=np.float32),
    }
    in_maps = []
    for c in range(n_cores):
        lo, hi = c * batches, (c + 1) * batches
        in_maps.append({
            "a_hl": np.ascontiguousarray(a_hl[lo:hi]),
            "b_t": np.ascontiguousarray(b_t[lo:hi]),
            **shared,
        })
    return in_maps


def postprocess(results, bc, n_cores=N_CORES, batches=BATCHES_PER_CORE):
    """results: list of per-core dicts with out_t [b, DF, seq] bf16 and
    z_d [b, P, NSUB] fp32 -> full [16, seq, DF] fp32 output."""
    bc = np.asarray(bc, np.float32)
    outs = []
    for r in results:
        ot = np.asarray(r["out_t"], np.float32)        # [b, DF, seq]
        z = np.asarray(r["z_d"], np.float32)           # [b, P, NSUB]
        for bi in range(ot.shape[0]):
            zb = z[bi].T.reshape(-1)[:ot.shape[2]]     # [seq]
            outs.append(ot[bi].T / zb[:, None] + bc)
    return np.ascontiguousarray(np.stack(outs, axis=0))


def kernel(input_a, input_b, Wa, ba, Wb, bb, Wc, bc):
    use_ba = bool(np.any(np.asarray(ba)))
    nc = _get_program(use_ba=use_ba)
    in_maps = make_in_maps(input_a, input_b, Wa, ba, Wb, bb, Wc, bc)
    res = run_bass_kernel_spmd(nc, in_maps, core_ids=list(range(N_CORES)))
    return postprocess(res.results, bc)


# revision 25
# speedup vs baseline: 1.1347x; 1.0211x over previous
"""Trainium2 Bass kernel for nn_CrossAttention (16x6209x256 cross-attention).

Strategy
--------
Data-parallel over batch: 16 batches -> 8 cores x 2 batches, pure SPMD.

Per batch:
    mapped_a = a @ Wa + ba            [6209, 64]
    mapped_b = b @ Wb + bb            [256, 64]
    scores   = mapped_a @ mapped_b.T * 8
    attn     = softmax(scores, -1)
    out      = (attn @ mapped_b) @ Wc + bc

Weights fold per batch (computed on device in exact fp32):
    Wfused    = 8 * Wa @ mapped_b.T               [256, 256]
    scoreBias = 8 * ba @ mapped_b.T               [256]
    Wout      = mapped_b @ Wc                     [256, 256]   (bc added on host)
    scores    = a @ Wfused + scoreBias
    out       = softmax(scores) @ Wout + bc

Numerics: scores run as a 3-term bf16 split (a = ahi+alo on host, Wfused =
Whi+Wlo on device): ahi@Whi + alo@Whi + ahi@Wlo.  Everything downstream of
exp runs in bf16 (attn, Wout, output) which costs ~6e-3 rel err against the
2e-2 budget and makes the PE transposes + final matmuls run at 1 cyc/row
with fast weight loads.

Softmax normalization is deferred to the host: exp() writes *unnormalized*
attn (bf16) and its row-sums z via the scalar-engine accumulator; the device
returns outT' = attnT_unnorm @ Wout (bf16) plus z, and the host computes
out = outT'.T / z + bc.  This removes the per-row normalize multiply and
halves the output DMA (bf16 instead of fp32).

The per-macro loop is software-pipelined: the scores matmuls for macro m+1
are issued before the softmax/transpose/final stage of macro m so the PE
never stalls waiting for the scalar-engine exp.

Layout: input_a transposed on host to [256, seq] (hi/lo bf16 stacked ->
[512, seq]); output produced transposed [256, seq] bf16, transposed back on
host. attn is transposed to [j, i] via PE transposes (bf16, vs identity).
z is accumulated as [128, n_subtiles] (i on partitions) and reshaped on host.
"""
import sys

for _p in ("/opt/trn_rl_repo",):
    if _p not in sys.path:
        sys.path.append(_p)

import numpy as np
import ml_dtypes

import concourse.bacc as bacc
import concourse.mybir as mybir
import concourse.tile as tile
from concourse.bass_utils import run_bass_kernel_spmd

F32 = mybir.dt.float32
BF16 = mybir.dt.bfloat16
I32 = mybir.dt.int32
P = 128

N_CORES = 8
BATCHES_PER_CORE = 2
SEQ = 6209
DF = 256          # feature dim of a / b
HID = 64          # projection dim
DMA_MACRO = 2048  # rows fetched/stored per DMA instruction
CMACRO = 512      # rows per compute macro (4 subtiles of 128)
NSUB = (SEQ + P - 1) // P  # 49 subtiles per batch (z columns)


def _row_plan(n_rows):
    """[(dma_start, dma_len, [(cm_start_within_dma, cm_len), ...]), ...]

    The first macro is small so compute can start before the bulk of the
    input has landed."""
    plan = []
    pos = 0
    while pos < n_rows:
        d = min(CMACRO if pos == 0 else DMA_MACRO, n_rows - pos)
        cms = []
        q = 0
        while q < d:
            c = min(CMACRO, d - q)
            cms.append((q, c))
            q += c
        plan.append((pos, d, cms))
        pos += d
    return plan


def build_program(seq=SEQ, batches=BATCHES_PER_CORE, use_ba=False):
    nc = bacc.Bacc("TRN2", target_bir_lowering=False, debug=False)

    a_hl = nc.dram_tensor("a_hl", [batches, 2 * DF, seq], BF16, kind="ExternalInput")
    b_t = nc.dram_tensor("b_t", [batches, DF, DF], F32, kind="ExternalInput")
    wat = nc.dram_tensor("wat", [HID, DF], F32, kind="ExternalInput")
    wb = nc.dram_tensor("wb", [DF, HID], F32, kind="ExternalInput")
    wc = nc.dram_tensor("wc", [HID, DF], F32, kind="ExternalInput")
    ba_d = nc.dram_tensor("ba_d", [HID, 1], F32, kind="ExternalInput")
    bb_d = nc.dram_tensor("bb_d", [HID, 1], F32, kind="ExternalInput")
    eye_d = nc.dram_tensor("eye_d", [P, P], BF16, kind="ExternalInput")
    ones_d = nc.dram_tensor("ones_d", [1, P], F32, kind="ExternalInput")
    out_t = nc.dram_tensor("out_t", [batches, DF, seq], BF16, kind="ExternalOutput")
    z_d = nc.dram_tensor("z_d", [batches, P, NSUB], F32, kind="ExternalOutput")

    Exp = mybir.ActivationFunctionType.Exp
    Copy = mybir.ActivationFunctionType.Copy
    Ident = mybir.ActivationFunctionType.Identity

    with tile.TileContext(nc) as tc:
        with (
            tc.tile_pool(name="const", bufs=1) as cpool,
            tc.tile_pool(name="wpool", bufs=2) as wpool,
            tc.tile_pool(name="zpool", bufs=2) as zpool,
            tc.tile_pool(name="apool", bufs=4) as apool,
            tc.tile_pool(name="mpool", bufs=3) as mpool,
            tc.tile_pool(name="opool", bufs=4) as opool,
            tc.tile_pool(name="pp", bufs=1, space="PSUM") as pp,
        ):
            # ---- per-core constants ----
            eye_sb = cpool.tile([P, P], BF16)
            nc.sync.dma_start(eye_sb[:], eye_d[:])
            wat_sb = cpool.tile([HID, DF], F32)
            nc.sync.dma_start(wat_sb[:], wat[:])
            wb_sb = cpool.tile([P, 2, HID], F32)
            nc.sync.dma_start(wb_sb[:], wb[:].rearrange("(k p) h -> p k h", p=P))
            wc_sb = cpool.tile([HID, DF], F32)
            nc.sync.dma_start(wc_sb[:], wc[:])
            ba_sb = cpool.tile([HID, 1], F32)
            nc.sync.dma_start(ba_sb[:], ba_d[:])
            bb_sb = cpool.tile([HID, 1], F32)
            nc.sync.dma_start(bb_sb[:], bb_d[:])
            ones_sb = cpool.tile([1, P], F32)
            nc.sync.dma_start(ones_sb[:], ones_d[:])

            # ---- per-batch fused weights (exact fp32 matmuls) ----
            whis, wlos, wos, sbiases, zsbs = [], [], [], [], []
            for b in range(batches):
                bT_sb = wpool.tile([P, 2, DF], F32, tag="bT")
                nc.sync.dma_start(bT_sb[:], b_t[b].rearrange("(k p) j -> p k j", p=P))

                ps_mb = pp.tile([HID, DF], F32, tag="fin0")
                for k in range(2):
                    nc.tensor.matmul(
                        ps_mb[:],
                        wb_sb[:, k, :],
                        bT_sb[:, k, :],
                        start=(k == 0), stop=(k == 1),
                    )
                mapped_bT = wpool.tile([HID, DF], F32, tag="mbT")
                nc.scalar.activation(mapped_bT[:], ps_mb[:], Ident, bias=bb_sb[:])

                # Wfused, split hi/lo into bf16 (scale 8 folded in)
                whi_sb = wpool.tile([P, 2, DF], BF16, tag="whi")
                wlo_sb = wpool.tile([P, 2, DF], BF16, tag="wlo")
                for c in range(2):
                    ps_wf = pp.tile([P, DF], F32, tag="fin0")
                    nc.tensor.matmul(
                        ps_wf[:],
                        wat_sb[:, c * P:(c + 1) * P],
                        mapped_bT[:],
                        start=True, stop=True,
                    )
                    nc.scalar.activation(whi_sb[:, c, :], ps_wf[:], Copy, scale=8.0)
                    # wlo = 8*wf - whi (rounded to bf16)
                    nc.vector.scalar_tensor_tensor(
                        wlo_sb[:, c, :],
                        ps_wf[:],
                        8.0,
                        whi_sb[:, c, :],
                        op0=mybir.AluOpType.mult,
                        op1=mybir.AluOpType.subtract,
                    )

                sbias_sb = None
                if use_ba:
                    ps_sbias = pp.tile([1, DF], F32, tag="fin1")
                    nc.tensor.matmul(
                        ps_sbias[:],
                        ba_sb[:],
                        mapped_bT[:],
                        start=True, stop=True,
                    )
                    sbias_sb = wpool.tile([1, DF], F32, tag="sbias")
                    nc.scalar.activation(sbias_sb[:], ps_sbias[:], Copy, scale=8.0)

                # Wout[j, f] = mapped_b @ Wc, as lhsT chunks [j(K), f(M)] in bf16
                wo_sb = wpool.tile([P, 2, DF], BF16, tag="wo")
                for c in range(2):
                    ps_wo = pp.tile([P, DF], F32, tag="fin1")
                    nc.tensor.matmul(
                        ps_wo[:],
                        mapped_bT[:, c * P:(c + 1) * P],
                        wc_sb[:],
                        start=True, stop=True,
                    )
                    nc.vector.tensor_copy(wo_sb[:, c, :], ps_wo[:])

                z_sb = zpool.tile([P, NSUB], F32, tag="z")
                whis.append(whi_sb)
                wlos.append(wlo_sb)
                wos.append(wo_sb)
                sbiases.append(sbias_sb)
                zsbs.append(z_sb)

            # ---- software-pipelined main loop over (batch, dma-macro, cmacro) ----
            # stage A (PE): scores matmuls into psum
            # stage B: max -> exp(bf16, accum z) -> transpose -> copies -> final
            def stage_a(b, aT_sb, mo, R, subs):
                whi_sb, wlo_sb = whis[b], wlos[b]
                scores_ps = pp.tile([P, 4 * DF], F32, tag="scores", bufs=2)
                for s, (io, r) in enumerate(subs):
                    c0 = s * DF
                    terms = []
                    for k in range(2):
                        ah = aT_sb[:, k, mo + io:mo + io + r]
                        al = aT_sb[:, 2 + k, mo + io:mo + io + r]
                        # ordered for stationary reuse: ah used twice in a row
                        terms += [
                            (ah, whi_sb[:, k, :]),
                            (ah, wlo_sb[:, k, :]),
                            (al, whi_sb[:, k, :]),
                        ]
                    # accumulate: k=0 triple then k=1 triple
                    terms = terms[0:2] + terms[3:5] + [terms[2], terms[5]]
                    for t, (lhs, rhs) in enumerate(terms):
                        nc.tensor.matmul(
                            scores_ps[:r, c0:c0 + DF],
                            lhs,
                            rhs,
                            start=(t == 0),
                            stop=(t == len(terms) - 1) and not use_ba,
                        )
                    if use_ba:
                        nc.tensor.matmul(
                            scores_ps[:r, c0:c0 + DF],
                            ones_sb[:, :r],
                            sbiases[b][:],
                            start=False, stop=True,
                        )
                return scores_ps

            def stage_b(b, scores_ps, d0, mo, R, subs):
                wo_sb, z_sb = wos[b], zsbs[b]
                rmax = max(r for _, r in subs)
                ns = len(subs)
                uniform = all(r == rmax for _, r in subs)

                negmax = mpool.tile([P, 4], F32, tag="negmax")
                if uniform:
                    nc.vector.tensor_reduce(
                        negmax[:rmax, :ns],
                        scores_ps[:rmax, :ns * DF].rearrange(
                            "p (s j) -> p s j", s=ns),
                        axis=mybir.AxisListType.X,
                        op=mybir.AluOpType.max,
                        negate=True,
                    )
                else:
                    for s, (io, r) in enumerate(subs):
                        nc.vector.tensor_reduce(
                            negmax[:r, s:s + 1],
                            scores_ps[:r, s * DF:(s + 1) * DF],
                            axis=mybir.AxisListType.X,
                            op=mybir.AluOpType.max,
                            negate=True,
                        )

                attn_sb = mpool.tile([P, 4 * DF], BF16, tag="attn")
                aT_ps = pp.tile([P, 2, CMACRO], BF16, tag="attnT", bufs=2)
                t_base = (d0 + mo) // P
                for s, (io, r) in enumerate(subs):
                    c0 = s * DF
                    nc.scalar.activation(
                        attn_sb[:r, c0:c0 + DF],
                        scores_ps[:r, c0:c0 + DF],
                        Exp,
                        bias=negmax[:r, s:s + 1],
                        accum_out=z_sb[:r, t_base + s:t_base + s + 1],
                    )
                    for jh in range(2):
                        nc.tensor.transpose(
                            aT_ps[:, jh, io:io + r],
                            attn_sb[:r, c0 + jh * P:c0 + (jh + 1) * P],
                            eye_sb[:r, :r],
                        )
                attnT = mpool.tile([P, 2, CMACRO], BF16, tag="attnTsb")
                if R % 2 == 0:
                    nc.vector.tensor_copy(
                        attnT[:, :, :R].bitcast(I32), aT_ps[:, :, :R].bitcast(I32))
                else:
                    nc.vector.tensor_copy(attnT[:, 0, :R], aT_ps[:, 0, :R])
                    nc.vector.tensor_copy(attnT[:, 1, :R], aT_ps[:, 1, :R])

                # final: outT[f, i] = sum_j Wout[j, f] attnT[j, i]
                outT_sb = opool.tile([P, 2, CMACRO], BF16, tag="outT")
                for c in range(2):
                    ps_fin = pp.tile([P, CMACRO], F32, tag=f"fin{c}")
                    for k in range(2):
                        nc.tensor.matmul(
                            ps_fin[:, :R],
                            wos[b][:, k, c * P:(c + 1) * P],
                            attnT[:, k, :R],
                            start=(k == 0), stop=(k == 1),
                        )
                    if c == 0:
                        nc.vector.tensor_copy(
                            outT_sb[:, c, :R], ps_fin[:, :R])
                    else:
                        nc.scalar.copy(
                            outT_sb[:, c, :R], ps_fin[:, :R])
                g0 = d0 + mo
                nc.sync.dma_start(
                    out_t[b][:, g0:g0 + R].rearrange("(c p) i -> p c i", p=P),
                    outT_sb[:, :, :R],
                )

            # build flat list of work items (b, d0, dlen, mo, R, subs)
            items = []
            for b in range(batches):
                for d0, dlen, cms in _row_plan(seq):
                    for mi, (mo, R) in enumerate(cms):
                        subs = [(o, min(P, R - o)) for o in range(0, R, P)]
                        items.append((b, d0, dlen, mo, R, subs,
                                      mi == 0, mi == len(cms) - 1))

            a_tiles = {}
            pending = None  # (item, scores_ps)
            for it in items:
                b, d0, dlen, mo, R, subs, first, last = it
                if first:
                    aT_sb = apool.tile([P, 4, DMA_MACRO], BF16, tag="aT")
                    nc.sync.dma_start(
                        aT_sb[:, :, :dlen],
                        a_hl[b][:, d0:d0 + dlen].rearrange(
                            "(k p) i -> p k i", p=P),
                    )
                    a_tiles[(b, d0)] = aT_sb
                sp = stage_a(b, a_tiles[(b, d0)], mo, R, subs)
                if pending is not None:
                    pb, pd0, pdlen, pmo, pR, psubs, _, _ = pending[0]
                    stage_b(pb, pending[1], pd0, pmo, pR, psubs)
                pending = (it, sp)
            # drain
            pb, pd0, pdlen, pmo, pR, psubs, _, _ = pending[0]
            stage_b(pb, pending[1], pd0, pmo, pR, psubs)
            for b in range(batches):
                nc.sync.dma_start(z_d[b], zsbs[b][:])

    nc.compile()
    return nc


_PROGRAM_CACHE = {}


def _get_program(seq=SEQ, batches=BATCHES_PER_CORE, use_ba=False):
    key = (seq, batches, use_ba)
    if key not in _PROGRAM_CACHE:
        _PROGRAM_CACHE[key] = build_program(seq, batches, use_ba)
    return _PROGRAM_CACHE[key]


def make_in_maps(input_a, input_b, Wa, ba, Wb, bb, Wc, bc,
                 n_cores=N_CORES, batches=BATCHES_PER_CORE):
    input_a = np.asarray(input_a, dtype=np.float32)
    input_b = np.asarray(input_b, dtype=np.float32)
    a_t = np.ascontiguousarray(input_a.transpose(0, 2, 1))      # [B, DF, seq]
    a_hi = a_t.astype(ml_dtypes.bfloat16)
    a_lo = (a_t - a_hi.astype(np.float32)).astype(ml_dtypes.bfloat16)
    # rows 0..DF-1 = hi, DF..2DF-1 = lo  -> [B, 2*DF, seq]
    a_hl = np.ascontiguousarray(np.concatenate([a_hi, a_lo], axis=1))
    b_t = np.ascontiguousarray(input_b.transpose(0, 2, 1))
    shared = {
        "wat": np.ascontiguousarray(np.asarray(Wa, np.float32).T),
        "wb": np.ascontiguousarray(np.asarray(Wb, np.float32)),
        "wc": np.ascontiguousarray(np.asarray(Wc, np.float32)),
        "ba_d": np.asarray(ba, np.float32).reshape(HID, 1).copy(),
        "bb_d": np.asarray(bb, np.float32).reshape(HID, 1).copy(),
        "eye_d": np.eye(P, dtype=ml_dtypes.bfloat16),
        "ones_d": np.ones((1, P), dtype=np.float32),
    }
    in_maps = []
    for c in range(n_cores):
        lo, hi = c * batches, (c + 1) * batches
        in_maps.append({
            "a_hl": np.ascontiguousarray(a_hl[lo:hi]),
            "b_t": np.ascontiguousarray(b_t[lo:hi]),
            **shared,
        })
    return in_maps


def postprocess(results, bc, n_cores=N_CORES, batches=BATCHES_PER_CORE):
    """results: list of per-core dicts with out_t [b, DF, seq] bf16 and
    z_d [b, P, NSUB] fp32 -> full [16, seq, DF] fp32 output."""
    bc = np.asarray(bc, np.float32)
    outs = []
    for r in results:
        ot = np.asarray(r["out_t"], np.float32)        # [b, DF, seq]
        z = np.asarray(r["z_d"], np.float32)           # [b, P, NSUB]
        for bi in range(ot.shape[0]):
            zb = z[bi].T.reshape(-1)[:ot.shape[2]]     # [seq]
            outs.append(ot[bi].T / zb[:, None] + bc)
    return np.ascontiguousarray(np.stack(outs, axis=0))


def kernel(input_a, input_b, Wa, ba, Wb, bb, Wc, bc):
    use_ba = bool(np.any(np.asarray(ba)))
    nc = _get_program(use_ba=use_ba)
    in_maps = make_in_maps(input_a, input_b, Wa, ba, Wb, bb, Wc, bc)
    res = run_bass_kernel_spmd(nc, in_maps, core_ids=list(range(N_CORES)))
    return postprocess(res.results, bc)
